# revision 1
# baseline (speedup 1.0000x reference)
"""GAT 2-layer kernel for trn2, 8 NeuronCores (SPMD).

Strategy (self-contained, hardcoded for N=100000, E=1600000, F=300):
 - nodes sharded contiguously across 8 cores (12500 each), degree-sorted
   within each core into 128-node tiles with a per-tile padded degree,
   consecutive tiles grouped into variable-size supertiles (ST tiles of
   common padded degree Gs, ST*Gs <= 80) so elementwise ops batch many
   tiles and amortize per-instruction overhead.  The tile/supertile
   profile is shared across cores so one SPMD program serves all 8.
 - 3 device launches, all dense DMA + PE/DVE/ACT compute:
     A: h1 = x @ W1, e_src/e_dst attention logits        -> [12544, 80]/core
     B: layer-1 edge softmax + weighted sum + b1 + ELU + W2aug -> [12544,66]
     C: layer-2 edge softmax + weighted sum + b2 + log_softmax -> [12544,64]
 - between launches the HOST performs the per-edge row gathers (pure index
   reordering into the layout the device streams densely; the HW indirect
   DMA paths measure ~215ns/row here which is unusable).  Softmax is
   computed without the segment-max shift (mathematically identical).
"""

import sys

sys.path.insert(0, "/opt/trn_rl_repo")

import numpy as np

import concourse.bass as bass
import concourse.bacc as bacc
import concourse.tile as tile
from concourse import mybir
from concourse.bass_utils import run_bass_kernel_spmd
from concourse.masks import make_identity

P = 128
NCORES = 8
N = 100000
F_IN = 300
FK = 384  # F_IN padded to 3*128 for matmul K-chunking
NPC = N // NCORES          # 12500 real nodes per core
NPAD = 12544               # padded to 98 tiles of 128
NT = NPAD // P             # 98 tiles
STG_BUDGET = 80            # max ST*Gs slots per partition per supertile
SENT_BIG = -60000.0        # e_src of the dummy table row (fp16-finite)

_cache = {}


# ---------------------------------------------------------------- host prep
def _host_prep(edge_index):
    src = np.asarray(edge_index[0], dtype=np.int64)
    dst = np.asarray(edge_index[1], dtype=np.int64)
    src = np.concatenate([src, np.arange(N, dtype=np.int64)])
    dst = np.concatenate([dst, np.arange(N, dtype=np.int64)])
    deg = np.bincount(dst, minlength=N)

    # CSR by dst
    order_e = np.argsort(dst, kind="stable")
    srcs_by_dst = src[order_e].astype(np.int64)
    row_ptr = np.zeros(N + 1, dtype=np.int64)
    np.cumsum(deg, out=row_ptr[1:])

    # per-core degree-sorted node order, padded with -1
    order_all = np.full((NCORES, NPAD), -1, dtype=np.int64)
    for c in range(NCORES):
        lo = c * NPC
        nodes = lo + np.argsort(deg[lo : lo + NPC], kind="stable")
        order_all[c, :NPC] = nodes

    # pi position of each node (row in the concatenated per-core shards)
    pos = np.empty(N + 1, dtype=np.int64)
    for c in range(NCORES):
        pos[order_all[c, :NPC]] = c * NPAD + np.arange(NPC)
    pos[N] = NCORES * NPAD  # sentinel -> dummy row appended to tables

    # shared per-tile padded degree (max over cores), even
    degp = np.zeros((NCORES, NPAD), dtype=np.int64)
    for c in range(NCORES):
        degp[c, :NPC] = deg[order_all[c, :NPC]]
    Gt = degp.reshape(NCORES, NT, P).max(axis=(0, 2))
    Gt = np.maximum(Gt + (Gt & 1), 2).astype(np.int64)

    # group consecutive tiles into supertiles with a common padded degree
    groups = []  # list of (start_tile, ST, Gs)
    t = 0
    while t < NT:
        g = int(Gt[t])
        st = 1
        while (t + st < NT and st < 8
               and (st + 1) * max(g, int(Gt[t + st])) <= STG_BUDGET):
            g = max(g, int(Gt[t + st]))
            st += 1
        groups.append((t, st, g))
        t += st

    # slot->table-position map: per supertile a [P, ST, Gs] block where
    # node (p, t) = order_all[c, (start+t)*P + p]
    tot_slots = int(sum(P * st * g for (_, st, g) in groups))
    A = np.full((NCORES, tot_slots), NCORES * NPAD, dtype=np.int64)
    pos_by_dst = pos[srcs_by_dst]
    for c in range(NCORES):
        off = 0
        for (t0, st, g) in groups:
            nodes = order_all[c, t0 * P : (t0 + st) * P].reshape(st, P).T
            safe = np.where(nodes >= 0, nodes, 0)
            k = np.where(nodes >= 0, deg[safe], 0)
            gi = np.arange(g)[None, None, :]
            mask = gi < k[:, :, None]
            src_idx = np.minimum(row_ptr[safe][:, :, None] + gi,
                                 len(pos_by_dst) - 1)
            blk = np.where(mask, pos_by_dst[src_idx], NCORES * NPAD)  # [P,st,g]
            A[c, off : off + P * st * g] = blk.ravel()
            off += P * st * g
    return order_all, pos, groups, A, tot_slots


# ------------------------------------------------------------- launch A prog
def _build_A():
    nc = bacc.Bacc(None, target_bir_lowering=False)
    f16 = mybir.dt.float16
    f32 = mybir.dt.float32
    xT = nc.dram_tensor("xT", [FK, NPAD], f16, kind="ExternalInput")
    w1 = nc.dram_tensor("w1", [FK, 64], f16, kind="ExternalInput")
    asrc = nc.dram_tensor("asrc", [64], f32, kind="ExternalInput")
    adst = nc.dram_tensor("adst", [64], f32, kind="ExternalInput")
    out = nc.dram_tensor("h1x", [NPAD, 80], f32, kind="ExternalOutput")

    with tile.TileContext(nc) as tc:
        with (
            tc.tile_pool(name="const", bufs=1) as cp,
            tc.tile_pool(name="xin", bufs=3) as xp,
            tc.tile_pool(name="work", bufs=3) as wp,
            tc.tile_pool(name="psum", bufs=3, space="PSUM") as pp,
        ):
            w1_t = cp.tile([P, 3, 64], f16)
            nc.sync.dma_start(
                out=w1_t[:], in_=w1[:, :].rearrange("(k p) n -> p k n", p=P)
            )
            asrc_t = cp.tile([P, 64], f32)
            nc.sync.dma_start(
                out=asrc_t[:],
                in_=bass.AP(tensor=asrc, offset=0, ap=[[0, P], [1, 64]]),
            )
            adst_t = cp.tile([P, 64], f32)
            nc.sync.dma_start(
                out=adst_t[:],
                in_=bass.AP(tensor=adst, offset=0, ap=[[0, P], [1, 64]]),
            )
            t0 = 0
            while t0 < NT:
                QT = min(8, NT - t0)
                xt = xp.tile([P, 3, 8 * P], f16, tag="x")
                nc.sync.dma_start(
                    out=xt[:, :, 0 : QT * P],
                    in_=bass.AP(
                        tensor=xT,
                        offset=t0 * P,
                        ap=[[NPAD, P], [NPAD * P, 3], [1, QT * P]],
                    ),
                )
                h_ps = pp.tile([P, 8 * 64], f32, tag="h")
                for tq in range(QT):
                    for k in range(3):
                        nc.tensor.matmul(
                            out=h_ps[:, tq * 64 : (tq + 1) * 64],
                            lhsT=xt[:, k, tq * P : (tq + 1) * P],
                            rhs=w1_t[:, k, :],
                            start=(k == 0),
                            stop=(k == 2),
                        )
                ot = wp.tile([P, 8 * 80], f32, tag="o")
                oap = ot[:]
                nc.scalar.copy(
                    out=bass.AP(tensor=oap.tensor, offset=oap.offset,
                                ap=[[oap.ap[0][0], P], [80, QT], [1, 64]]),
                    in_=h_ps[:, 0 : QT * 64],
                )
                tmp = wp.tile([P, 8 * 64], f32, tag="tmp")
                nc.vector.tensor_tensor(
                    out=tmp[:, 0 : QT * 64], in0=h_ps[:, 0 : QT * 64],
                    in1=_ap(asrc_t[:], 0, [[0, QT], [1, 64]]),
                    op=mybir.AluOpType.mult,
                )
                nc.vector.reduce_sum(
                    out=bass.AP(tensor=oap.tensor, offset=oap.offset + 64,
                                ap=[[oap.ap[0][0], P], [80, QT], [1, 8]]),
                    in_=tmp[:, 0 : QT * 64].rearrange("p (q h d) -> p q h d", h=8, d=8),
                    axis=mybir.AxisListType.X,
                )
                nc.vector.tensor_tensor(
                    out=tmp[:, 0 : QT * 64], in0=h_ps[:, 0 : QT * 64],
                    in1=_ap(adst_t[:], 0, [[0, QT], [1, 64]]),
                    op=mybir.AluOpType.mult,
                )
                nc.vector.reduce_sum(
                    out=bass.AP(tensor=oap.tensor, offset=oap.offset + 72,
                                ap=[[oap.ap[0][0], P], [80, QT], [1, 8]]),
                    in_=tmp[:, 0 : QT * 64].rearrange("p (q h d) -> p q h d", h=8, d=8),
                    axis=mybir.AxisListType.X,
                )
                nc.sync.dma_start(
                    out=bass.AP(tensor=out, offset=t0 * P * 80,
                                ap=[[80, P], [80 * P, QT], [1, 80]]),
                    in_=ot[:, 0 : QT * 80].rearrange("p (q c) -> p q c", c=80),
                )
                t0 += QT
    nc.finalize()
    return nc


def _ap(base_ap, off, dims):
    return bass.AP(tensor=base_ap.tensor, offset=base_ap.offset + off,
                   ap=[[base_ap.ap[0][0], P]] + dims)


# ------------------------------------------------------------- launch B prog
def _build_B(groups):
    """Layer-1 edge pass + b1 + ELU + W2aug matmul -> g2 rows [NPAD, 66]."""
    nc = bacc.Bacc(None, target_bir_lowering=False)
    f16 = mybir.dt.float16
    f32 = mybir.dt.float32
    tot = int(sum(P * st * g for (_, st, g) in groups))
    ge = nc.dram_tensor("ge", [tot * 72], f16, kind="ExternalInput")
    edst = nc.dram_tensor("edst", [NPAD, 8], f32, kind="ExternalInput")
    b1 = nc.dram_tensor("b1", [64], f32, kind="ExternalInput")
    w2aug = nc.dram_tensor("w2aug", [64, 66], f32, kind="ExternalInput")
    badj = nc.dram_tensor("badj", [66], f32, kind="ExternalInput")
    out = nc.dram_tensor("g2", [NPAD, 66], f32, kind="ExternalOutput")

    AT = mybir.ActivationFunctionType
    OP = mybir.AluOpType
    with tile.TileContext(nc) as tc:
        with (
            tc.tile_pool(name="const", bufs=1) as cp,
            tc.tile_pool(name="gin", bufs=3) as gp,
            tc.tile_pool(name="work", bufs=2) as wp,
            tc.tile_pool(name="outp", bufs=3) as op_,
            tc.tile_pool(name="psum", bufs=4, space="PSUM") as pp,
        ):
            iden = cp.tile([P, P], f32)
            make_identity(nc, iden[:])
            edst_t = cp.tile([P, NT * 8], f32)
            nc.sync.dma_start(
                out=edst_t[:],
                in_=bass.AP(tensor=edst, offset=0,
                            ap=[[8, P], [8 * P, NT], [1, 8]]),
            )
            b1_t = cp.tile([P, 64], f32)
            nc.sync.dma_start(
                out=b1_t[:],
                in_=bass.AP(tensor=b1, offset=0, ap=[[0, P], [1, 64]]),
            )
            w2_t = cp.tile([64, 66], f32)
            nc.sync.dma_start(out=w2_t[:], in_=w2aug[:, :])
            badj_t = cp.tile([P, 66], f32)
            nc.sync.dma_start(
                out=badj_t[:],
                in_=bass.AP(tensor=badj, offset=0, ap=[[0, P], [1, 66]]),
            )
            off = 0
            for (t0, st, g) in groups:
                R = st * g          # slots per partition in this supertile
                gt = gp.tile([P, R * 72], f16, tag="g")
                nc.sync.dma_start(
                    out=gt[:],
                    in_=bass.AP(tensor=ge, offset=off,
                                ap=[[R * 72, P], [1, R * 72]]),
                )
                off += P * R * 72
                gv = gt[:]
                # e_sum[p, t, h*g+gi] = ge_esrc[p, t, gi, h] + edst[p, t0+t, h]
                es = wp.tile([P, 8 * R], f32, tag="es")
                nc.gpsimd.tensor_tensor(
                    out=_ap(es[:], 0, [[g, st], [1, g], [R, 8]]),
                    in0=_ap(gv, 64, [[g * 72, st], [72, g], [1, 8]]),
                    in1=_ap(edst_t[:], t0 * 8, [[8, st], [0, g], [1, 8]]),
                    op=OP.add,
                )
                w_t = wp.tile([P, 8 * R], f32, tag="w")
                nc.vector.scalar_tensor_tensor(out=w_t[:], in0=es[:], scalar=0.2,
                                               in1=es[:], op0=OP.mult, op1=OP.max)
                nc.scalar.activation(out=w_t[:], in_=w_t[:], func=AT.Exp)
                den = wp.tile([P, 8 * st], f32, tag="den")
                nc.vector.reduce_sum(
                    out=den[:],
                    in_=_ap(w_t[:], 0, [[g, st], [R, 8], [1, g]]),
                    axis=mybir.AxisListType.X,
                )
                nc.vector.tensor_scalar_max(out=den[:], in0=den[:], scalar1=1e-30)
                rec = wp.tile([P, 8 * st], f32, tag="rec")
                nc.vector.reciprocal(out=rec[:], in_=den[:])
                # ws[p, t, (h*8+d)*g+gi] = ge_h[p, t, gi, h, d] * w[p, t, h, gi]
                # (TensorTensor ISA is limited to 3 free dims -> per-tile loop)
                ws = wp.tile([P, 64 * R], f32, tag="ws")
                o1 = wp.tile([P, 64 * st], f32, tag="o1")
                # ws[p, (h*8+d)*R + t*g + gi]; supertile-wide 3-dim ops
                nc.vector.tensor_tensor(
                    out=_ap(ws[:], 0, [[1, R], [8 * R, 4], [R, 8]]),
                    in0=_ap(gv, 0, [[72, R], [8, 4], [1, 8]]),
                    in1=_ap(w_t[:], 0, [[1, R], [R, 4], [0, 8]]),
                    op=OP.mult,
                )
                nc.gpsimd.tensor_tensor(
                    out=_ap(ws[:], 32 * R, [[1, R], [8 * R, 4], [R, 8]]),
                    in0=_ap(gv, 32, [[72, R], [8, 4], [1, 8]]),
                    in1=_ap(w_t[:], 4 * R, [[1, R], [R, 4], [0, 8]]),
                    op=OP.mult,
                )
                nc.vector.reduce_sum(
                    out=o1[:],
                    in_=_ap(ws[:], 0, [[g, st], [R, 64], [1, g]]),
                    axis=mybir.AxisListType.X,
                )
                nc.vector.tensor_tensor(
                    out=o1[:], in0=o1[:],
                    in1=_ap(rec[:], 0, [[8, st], [1, 8], [0, 8]]),
                    op=OP.mult,
                )
                nc.vector.tensor_tensor(
                    out=o1[:], in0=o1[:],
                    in1=_ap(b1_t[:], 0, [[0, st], [1, 64]]),
                    op=OP.add,
                )
                # z' = relu(zp) + exp(min(zp,0))  (= elu(zp)+1)
                m = wp.tile([P, 64 * st], f32, tag="m")
                nc.vector.tensor_scalar_min(out=m[:], in0=o1[:], scalar1=0.0)
                nc.scalar.activation(out=m[:], in_=m[:], func=AT.Exp)
                z1 = wp.tile([P, 64 * st], f32, tag="z1")
                nc.vector.scalar_tensor_tensor(
                    out=z1[:], in0=o1[:], scalar=0.0, in1=m[:],
                    op0=OP.max, op1=OP.add,
                )
                g2t = op_.tile([P, 66 * st], f32, tag="g2t")
                for t in range(st):
                    zT_ps = pp.tile([64, P], f32, tag="zT")
                    nc.tensor.transpose(out=zT_ps[:],
                                        in_=z1[:, t * 64:(t + 1) * 64],
                                        identity=iden[:])
                    zT = wp.tile([64, P], f32, tag="zTs")
                    nc.scalar.copy(out=zT[:], in_=zT_ps[:])
                    h2_ps = pp.tile([P, 66], f32, tag="h2")
                    nc.tensor.matmul(out=h2_ps[:], lhsT=zT[:], rhs=w2_t[:],
                                     start=True, stop=True)
                    nc.vector.tensor_tensor(out=g2t[:, t * 66:(t + 1) * 66],
                                            in0=h2_ps[:], in1=badj_t[:],
                                            op=OP.add)
                nc.sync.dma_start(
                    out=bass.AP(tensor=out, offset=t0 * P * 66,
                                ap=[[66, P], [66 * P, st], [1, 66]]),
                    in_=g2t[:].rearrange("p (t c) -> p t c", c=66),
                )
    nc.finalize()
    return nc


# ------------------------------------------------------------- launch C prog
def _build_C(groups):
    """Layer-2 edge pass + b2 + log_softmax -> [NPAD, 64]."""
    nc = bacc.Bacc(None, target_bir_lowering=False)
    f16 = mybir.dt.float16
    f32 = mybir.dt.float32
    tot = int(sum(P * st * g for (_, st, g) in groups))
    ge = nc.dram_tensor("ge", [tot * 66], f16, kind="ExternalInput")
    edst = nc.dram_tensor("edst", [NPAD], f32, kind="ExternalInput")
    b2 = nc.dram_tensor("b2", [64], f32, kind="ExternalInput")
    out = nc.dram_tensor("res", [NPAD, 64], f32, kind="ExternalOutput")

    AT = mybir.ActivationFunctionType
    OP = mybir.AluOpType
    with tile.TileContext(nc) as tc:
        with (
            tc.tile_pool(name="const", bufs=1) as cp,
            tc.tile_pool(name="gin", bufs=3) as gp,
            tc.tile_pool(name="work", bufs=2) as wp,
            tc.tile_pool(name="outp", bufs=3) as op_,
        ):
            edst_t = cp.tile([P, NT], f32)
            nc.sync.dma_start(
                out=edst_t[:],
                in_=bass.AP(tensor=edst, offset=0, ap=[[1, P], [P, NT]]),
            )
            b2_t = cp.tile([P, 64], f32)
            nc.sync.dma_start(
                out=b2_t[:],
                in_=bass.AP(tensor=b2, offset=0, ap=[[0, P], [1, 64]]),
            )
            off = 0
            for (t0, st, g) in groups:
                R = st * g
                gt = gp.tile([P, R * 66], f16, tag="g")
                nc.sync.dma_start(
                    out=gt[:],
                    in_=bass.AP(tensor=ge, offset=off,
                                ap=[[R * 66, P], [1, R * 66]]),
                )
                off += P * R * 66
                gv = gt[:]
                es = wp.tile([P, R], f32, tag="es")
                nc.gpsimd.tensor_tensor(
                    out=es[:],
                    in0=_ap(gv, 64, [[g * 66, st], [66, g]]),
                    in1=_ap(edst_t[:], t0, [[1, st], [0, g]]),
                    op=OP.add,
                )
                w_t = wp.tile([P, R], f32, tag="w")
                nc.vector.scalar_tensor_tensor(out=w_t[:], in0=es[:], scalar=0.2,
                                               in1=es[:], op0=OP.mult, op1=OP.max)
                nc.scalar.activation(out=w_t[:], in_=w_t[:], func=AT.Exp)
                den = wp.tile([P, st], f32, tag="den")
                nc.vector.reduce_sum(
                    out=den[:],
                    in_=_ap(w_t[:], 0, [[g, st], [1, g]]),
                    axis=mybir.AxisListType.X,
                )
                nc.vector.tensor_scalar_max(out=den[:], in0=den[:], scalar1=1e-30)
                rec = wp.tile([P, st], f32, tag="rec")
                nc.vector.reciprocal(out=rec[:], in_=den[:])
                ws = wp.tile([P, 64 * R], f32, tag="ws")
                nc.vector.tensor_tensor(
                    out=_ap(ws[:], 0, [[64 * g, st], [1, g], [g, 24]]),
                    in0=_ap(gv, 0, [[g * 66, st], [66, g], [1, 24]]),
                    in1=_ap(w_t[:], 0, [[g, st], [1, g], [0, 24]]),
                    op=OP.mult,
                )
                nc.gpsimd.tensor_tensor(
                    out=_ap(ws[:], 24 * g, [[64 * g, st], [1, g], [g, 40]]),
                    in0=_ap(gv, 24, [[g * 66, st], [66, g], [1, 40]]),
                    in1=_ap(w_t[:], 0, [[g, st], [1, g], [0, 40]]),
                    op=OP.mult,
                )
                o1 = wp.tile([P, 64 * st], f32, tag="o1")
                nc.vector.reduce_sum(
                    out=o1[:],
                    in_=_ap(ws[:], 0, [[64 * g, st], [g, 64], [1, g]]),
                    axis=mybir.AxisListType.X,
                )
                z = wp.tile([P, 64 * st], f32, tag="z")
                nc.vector.tensor_tensor(
                    out=z[:], in0=o1[:],
                    in1=_ap(rec[:], 0, [[1, st], [0, 64]]),
                    op=OP.mult,
                )
                nc.vector.tensor_tensor(
                    out=z[:], in0=z[:],
                    in1=_ap(b2_t[:], 0, [[0, st], [1, 64]]),
                    op=OP.add,
                )
                # log_softmax per 64-wide block; z is bounded (~+-12) so the
                # max-shift is unnecessary in fp32: out = z - ln(sum(exp(z)))
                ex = wp.tile([P, 64 * st], f32, tag="ex")
                nc.scalar.activation(out=ex[:], in_=z[:], func=AT.Exp)
                ssum = wp.tile([P, st], f32, tag="ssum")
                nc.vector.reduce_sum(
                    out=ssum[:],
                    in_=ex[:].rearrange("p (t c) -> p t c", c=64),
                    axis=mybir.AxisListType.X,
                )
                lse = wp.tile([P, st], f32, tag="lse")
                nc.scalar.activation(out=lse[:], in_=ssum[:], func=AT.Ln)
                ot = op_.tile([P, 64 * st], f32, tag="ot")
                nc.gpsimd.tensor_tensor(
                    out=ot[:], in0=z[:],
                    in1=_ap(lse[:], 0, [[1, st], [0, 64]]),
                    op=OP.subtract,
                )
                nc.sync.dma_start(
                    out=bass.AP(tensor=out, offset=t0 * P * 64,
                                ap=[[64, P], [64 * P, st], [1, 64]]),
                    in_=ot[:].rearrange("p (t c) -> p t c", c=64),
                )
    nc.finalize()
    return nc


# ------------------------------------------------------------------- driver
def _get_programs(groups):
    key = tuple(groups)
    if key not in _cache:
        _cache[key] = (_build_A(), _build_B(groups), _build_C(groups))
    return _cache[key]


def kernel(x, edge_index, W1, att_src1, att_dst1, b1, W2, att_src2, att_dst2, b2,
           _timings=None):
    import time as _time

    x = np.asarray(x, dtype=np.float32)
    W1 = np.asarray(W1, dtype=np.float32)
    order_all, pos, groups, A, tot = _host_prep(np.asarray(edge_index))
    ncA, ncB, ncC = _get_programs(groups)

    # ---- launch A inputs
    w1p = np.zeros((FK, 64), np.float32)
    w1p[:F_IN] = W1
    asrc = np.asarray(att_src1, np.float32).ravel()
    adst = np.asarray(att_dst1, np.float32).ravel()
    xpad = np.vstack([x, np.zeros((1, F_IN), np.float32)])
    in_A = []
    for c in range(NCORES):
        xa = xpad[np.where(order_all[c] >= 0, order_all[c], N)]  # [NPAD, 300]
        xT = np.zeros((FK, NPAD), np.float16)
        xT[:F_IN] = xa.T
        in_A.append({"xT": xT, "w1": w1p.astype(np.float16), "asrc": asrc,
                     "adst": adst})

    t0 = _time.perf_counter()
    resA = run_bass_kernel_spmd(ncA, in_A, core_ids=list(range(NCORES)))
    tA = _time.perf_counter() - t0

    h1x = np.concatenate([r["h1x"] for r in resA.results], axis=0)  # [8*NPAD,80]
    tab1 = np.vstack([h1x[:, :72].astype(np.float16),
                      np.zeros((1, 72), np.float16)])
    tab1[-1, 64:72] = SENT_BIG

    # ---- launch B inputs
    W2 = np.asarray(W2, np.float32)
    w2aug = np.concatenate(
        [W2, (W2 @ np.asarray(att_src2, np.float32).ravel())[:, None],
         (W2 @ np.asarray(att_dst2, np.float32).ravel())[:, None]], axis=1)
    badj = -w2aug.sum(axis=0).astype(np.float32)
    b1 = np.asarray(b1, np.float32)
    in_B = []
    for c in range(NCORES):
        ge = tab1[A[c]].ravel()
        in_B.append({"ge": ge,
                     "edst": h1x[c * NPAD:(c + 1) * NPAD, 72:80].copy(),
                     "b1": b1, "w2aug": w2aug, "badj": badj})

    t0 = _time.perf_counter()
    resB = run_bass_kernel_spmd(ncB, in_B, core_ids=list(range(NCORES)))
    tB = _time.perf_counter() - t0

    g2 = np.concatenate([r["g2"] for r in resB.results], axis=0)  # [8*NPAD, 66]
    tab2 = np.vstack([g2[:, :66].astype(np.float16),
                      np.zeros((1, 66), np.float16)])
    tab2[-1, 64] = SENT_BIG

    # ---- launch C inputs
    b2 = np.asarray(b2, np.float32)
    in_C = []
    for c in range(NCORES):
        ge = tab2[A[c]].ravel()
        in_C.append({"ge": ge, "edst": g2[c * NPAD:(c + 1) * NPAD, 65].copy(),
                     "b2": b2})

    t0 = _time.perf_counter()
    resC = run_bass_kernel_spmd(ncC, in_C, core_ids=list(range(NCORES)))
    tC = _time.perf_counter() - t0

    out = np.empty((N, 64), np.float32)
    for c in range(NCORES):
        valid = order_all[c] >= 0
        out[order_all[c][valid]] = resC.results[c]["res"][valid]
    if _timings is not None:
        _timings.update({"A": tA, "B": tB, "C": tC})
    return out



# revision 42
# speedup vs baseline: 2.0757x; 2.0757x over previous
"""GAT 2-layer kernel for trn2, 8 NeuronCores (SPMD).

Strategy (self-contained, hardcoded for N=100000, E=1600000, F=300):
 - nodes sharded contiguously across 8 cores (12500 each), degree-sorted
   within each core into 128-node tiles with a per-tile padded degree,
   consecutive tiles grouped into supertiles (st tiles of common padded
   degree g, st*g <= 80); the profile is shared across cores so one SPMD
   program serves all 8.
 - 3 device launches, all dense DMA + fp16 compute:
     A: h1x = x @ [W1 | W1@Asrc | W1@Adst]  -> [P, T*80] per core
     B: layer-1 edge softmax + weighted sum + b1 + ELU + W2aug -> [P, T*66]
     C: layer-2 edge softmax + weighted sum + b2 + log_softmax -> [P, T*64]
 - between launches the HOST performs the per-edge row gathers (pure index
   reordering into the layout the device streams densely).
 - the per-edge weighted sum runs as PE matmuls against a constant fp16
   identity (PSUM-accumulated copies), with the softmax denominator carried
   as extra ws columns; the alpha*h multiply runs on DVE in fp16 2x mode
   using a pair-expanded weight vector so every operand stays packed.
"""

import sys

sys.path.insert(0, "/opt/trn_rl_repo")

import numpy as np

import concourse.bass as bass
import concourse.bacc as bacc
import concourse.tile as tile
from concourse import mybir
from concourse.bass_utils import run_bass_kernel_spmd
from concourse.masks import make_identity

P = 128
NCORES = 8
N = 100000
F_IN = 300
NPC = N // NCORES          # 12500 real nodes per core
NPAD = 12544               # padded to 98 tiles of 128
NT = NPAD // P             # 98 tiles
STG_BUDGET = 80            # max st*g slots per partition per supertile
SENT_BIG = -60000.0        # e_src of the dummy table row (fp16-finite)

_cache = {}


# ---------------------------------------------------------------- host prep
def _host_prep(edge_index):
    src = np.asarray(edge_index[0], dtype=np.int64)
    dst = np.asarray(edge_index[1], dtype=np.int64)
    src = np.concatenate([src, np.arange(N, dtype=np.int64)])
    dst = np.concatenate([dst, np.arange(N, dtype=np.int64)])
    deg = np.bincount(dst, minlength=N)

    # CSR by dst
    order_e = np.argsort(dst, kind="stable")
    srcs_by_dst = src[order_e].astype(np.int64)
    row_ptr = np.zeros(N + 1, dtype=np.int64)
    np.cumsum(deg, out=row_ptr[1:])

    # per-core degree-sorted node order, padded with -1
    order_all = np.full((NCORES, NPAD), -1, dtype=np.int64)
    for c in range(NCORES):
        lo = c * NPC
        nodes = lo + np.argsort(deg[lo : lo + NPC], kind="stable")
        order_all[c, :NPC] = nodes

    # pi position of each node (row in the concatenated per-core shards)
    pos = np.empty(N + 1, dtype=np.int64)
    for c in range(NCORES):
        pos[order_all[c, :NPC]] = c * NPAD + np.arange(NPC)
    pos[N] = NCORES * NPAD  # sentinel -> dummy row appended to tables

    # shared per-tile padded degree (max over cores), even
    degp = np.zeros((NCORES, NPAD), dtype=np.int64)
    for c in range(NCORES):
        degp[c, :NPC] = deg[order_all[c, :NPC]]
    Gt = degp.reshape(NCORES, NT, P).max(axis=(0, 2))
    Gt = np.maximum(Gt + (Gt & 1), 2).astype(np.int64)

    # group consecutive tiles into supertiles with a common padded degree
    groups = []  # list of (start_tile, st, g)
    t = 0
    while t < NT:
        g = int(Gt[t])
        st = 1
        while (t + st < NT and st < 6
               and (st + 1) * max(g, int(Gt[t + st])) <= STG_BUDGET):
            g = max(g, int(Gt[t + st]))
            st += 1
        groups.append((t, st, g))
        t += st


    # slot->table-position map: per supertile a [P, st, g] block where
    # node (p, t) = order_all[c, (start+t)*P + p]
    tot_slots = int(sum(P * st * g for (_, st, g) in groups))
    A = np.full((NCORES, tot_slots), NCORES * NPAD, dtype=np.int64)
    pos_by_dst = pos[srcs_by_dst]
    for c in range(NCORES):
        off = 0
        for (t0, st, g) in groups:
            nodes = order_all[c, t0 * P : (t0 + st) * P].reshape(st, P).T
            safe = np.where(nodes >= 0, nodes, 0)
            k = np.where(nodes >= 0, deg[safe], 0)
            gi = np.arange(g)[None, None, :]
            mask = gi < k[:, :, None]
            src_idx = np.minimum(row_ptr[safe][:, :, None] + gi,
                                 len(pos_by_dst) - 1)
            blk = np.where(mask, pos_by_dst[src_idx], NCORES * NPAD)  # [P,st,g]
            A[c, off : off + P * st * g] = blk.ravel()
            off += P * st * g
    return order_all, pos, groups, A, tot_slots


def _ap(base_ap, off, dims):
    return bass.AP(tensor=base_ap.tensor, offset=base_ap.offset + off,
                   ap=[[base_ap.ap[0][0], base_ap.ap[0][1]]] + dims)


# ------------------------------------------------------------- launch A prog
def _build_A():
    """h1x[P, T*80] = (x @ [W1 | W1@Asrc | W1@Adst]).T-tiled, all fp16."""
    nc = bacc.Bacc(None, target_bir_lowering=False)
    f16 = mybir.dt.float16
    f32 = mybir.dt.float32
    xT = nc.dram_tensor("xT", [F_IN, NPAD], f16, kind="ExternalInput")
    w1 = nc.dram_tensor("w1", [F_IN, 80], f16, kind="ExternalInput")
    out = nc.dram_tensor("h1x", [P, NT * 80], f16, kind="ExternalOutput")

    QT = 24   # tiles per DMA round (4 PSUM sub-batches of 6)
    PQ = 6    # tiles per PSUM tile (6*80*4B = 1920B, fits one bank)
    with tile.TileContext(nc) as tc:
        with (
            tc.tile_pool(name="const", bufs=1) as cp,
            tc.tile_pool(name="xin", bufs=3) as xp,
            tc.tile_pool(name="work", bufs=3) as wp,
            tc.tile_pool(name="psum", bufs=4, space="PSUM") as pp,
        ):
            w1a = cp.tile([P, 80], f16, tag="w1a")
            nc.sync.dma_start(out=w1a[:], in_=w1[0:P, :])
            w1b = cp.tile([P, 80], f16, tag="w1b")
            nc.sync.dma_start(out=w1b[:], in_=w1[P : 2 * P, :])
            w1c = cp.tile([P, 80], f16, tag="w1c")
            nc.sync.dma_start(out=w1c[0:44, :], in_=w1[2 * P : F_IN, :])
            t0 = 0
            for q in (6, 24, 24, 24, 20):
                xt = xp.tile([P, 2, QT * P], f16, tag="x")
                nc.sync.dma_start(
                    out=xt[:, :, 0 : q * P],
                    in_=bass.AP(
                        tensor=xT, offset=t0 * P,
                        ap=[[NPAD, P], [NPAD * P, 2], [1, q * P]],
                    ),
                )
                xt2 = xp.tile([P, QT * P], f16, tag="x2")
                nc.sync.dma_start(
                    out=xt2[0:44, 0 : q * P],
                    in_=bass.AP(
                        tensor=xT, offset=2 * P * NPAD + t0 * P,
                        ap=[[NPAD, 44], [1, q * P]],
                    ),
                )
                ot = wp.tile([P, QT * 80], f16, tag="o")
                for j in range(0, q, PQ):
                    jq = min(PQ, q - j)
                    h_ps = pp.tile([P, PQ * 80], f32, tag="h")
                    for ti in range(jq):
                        tq = j + ti
                        nc.tensor.matmul(
                            out=h_ps[:, ti * 80 : (ti + 1) * 80],
                            lhsT=xt[:, 0, tq * P : (tq + 1) * P],
                            rhs=w1a[:], start=True, stop=False,
                        )
                        nc.tensor.matmul(
                            out=h_ps[:, ti * 80 : (ti + 1) * 80],
                            lhsT=xt[:, 1, tq * P : (tq + 1) * P],
                            rhs=w1b[:], start=False, stop=False,
                        )
                        nc.tensor.matmul(
                            out=h_ps[:, ti * 80 : (ti + 1) * 80],
                            lhsT=xt2[0:44, tq * P : (tq + 1) * P],
                            rhs=w1c[0:44, :], start=False, stop=True,
                        )
                    nc.scalar.copy(out=ot[:, j * 80 : (j + jq) * 80],
                                   in_=h_ps[:, 0 : jq * 80])
                nc.sync.dma_start(
                    out=bass.AP(tensor=out, offset=t0 * 80,
                                ap=[[NT * 80, P], [1, q * 80]]),
                    in_=ot[:, 0 : q * 80],
                )
                t0 += q
    nc.finalize()
    return nc


# ------------------------------------------------------------- launch B prog
def _build_B(groups):
    """Layer-1 edge pass + b1 + ELU + W2aug matmul -> g2 [P, T*66] fp16."""
    nc = bacc.Bacc(None, target_bir_lowering=False)
    f16 = mybir.dt.float16
    f32 = mybir.dt.float32
    tot = int(sum(P * st * g for (_, st, g) in groups))
    geh = nc.dram_tensor("geh", [tot * 64], f16, kind="ExternalInput")
    gee = nc.dram_tensor("gee", [tot * 8], f16, kind="ExternalInput")
    edst = nc.dram_tensor("edst", [P, NT * 8], f16, kind="ExternalInput")
    w2aug = nc.dram_tensor("w2aug", [64, 66], f16, kind="ExternalInput")
    out = nc.dram_tensor("g2", [P, NT * 66], f16, kind="ExternalOutput")

    AT = mybir.ActivationFunctionType
    OP = mybir.AluOpType
    with tile.TileContext(nc) as tc:
        with (
            tc.tile_pool(name="const", bufs=1) as cp,
            tc.tile_pool(name="gin", bufs=6) as gp,
            tc.tile_pool(name="work", bufs=4) as wp,
            tc.tile_pool(name="outp", bufs=4) as op_,
            tc.tile_pool(name="psum", bufs=2, space="PSUM") as pp,
            tc.tile_pool(name="psumt", bufs=4, space="PSUM") as pt,
        ):
            iden = cp.tile([P, P], f16, tag="iden")
            make_identity(nc, iden[:])
            edst_t = cp.tile([P, NT * 8], f16, tag="edst")
            nc.sync.dma_start(
                out=edst_t[:],
                in_=bass.AP(tensor=edst, offset=0,
                            ap=[[NT * 8, P], [1, NT * 8]]),
            )
            w2_t = cp.tile([64, 66], f16, tag="w2")
            nc.sync.dma_start(out=w2_t[:], in_=w2aug[:, :])

            offs = []
            oh = oe = 0
            for (t0, st, g) in groups:
                offs.append((oh, oe))
                oh += P * st * g * 64
                oe += P * st * g * 8
            state = {}

            def s0(i):
                """DMA in + edge logits + softmax weights."""
                (t0, st, g) = groups[i]
                R = st * g
                off_h, off_e = offs[i]
                gee_t = gp.tile([P, STG_BUDGET * 8], f16, tag="ge")
                nc.sync.dma_start(
                    out=gee_t[:, 0 : R * 8],
                    in_=bass.AP(tensor=gee, offset=off_e,
                                ap=[[R * 8, P], [1, R * 8]]),
                )
                geh_t = gp.tile([P, STG_BUDGET * 64], f16, tag="gh")
                nc.sync.dma_start(
                    out=geh_t[:, 0 : R * 64],
                    in_=bass.AP(tensor=geh, offset=off_h,
                                ap=[[R * 64, P], [1, R * 64]]),
                )
                # es[p, t, gi, h] = gee[p, t, gi, h] + edst[p, t0+t, h]
                es = wp.tile([P, STG_BUDGET * 8], f16, tag="es")
                nc.gpsimd.tensor_tensor(
                    out=_ap(es[:], 0, [[8 * g, st], [8, g], [1, 8]]),
                    in0=_ap(gee_t[:], 0, [[8 * g, st], [8, g], [1, 8]]),
                    in1=_ap(edst_t[:], t0 * 8, [[8, st], [0, g], [1, 8]]),
                    op=OP.add,
                )
                lr = wp.tile([P, STG_BUDGET * 8], f16, tag="lr")
                nc.scalar.activation(out=lr[:, 0 : 8 * R],
                                     in_=es[:, 0 : 8 * R],
                                     func=AT.Prelu, alpha=0.2)
                # ws cols 64..72 <- w = exp(lr), h-interleaved per slot
                ws = wp.tile([P, STG_BUDGET * 72], f16, tag="ws")
                nc.scalar.activation(
                    out=_ap(ws[:], 64, [[72, R], [1, 8]]),
                    in_=_ap(lr[:], 0, [[8, R], [1, 8]]),
                    func=AT.Exp,
                )
                # wx[p, s, h, j] = w (pair-expanded for packed 2x broadcast)
                wx = wp.tile([P, STG_BUDGET * 16], f16, tag="wx")
                nc.scalar.activation(
                    out=_ap(wx[:], 0, [[16, R], [2, 8], [1, 2]]),
                    in_=_ap(lr[:], 0, [[8, R], [1, 8], [0, 2]]),
                    func=AT.Exp,
                )
                state[i] = [geh_t, ws, wx]

            def s1(i):
                """alpha*h multiply + PE reduce."""
                (t0, st, g) = groups[i]
                R = st * g
                geh_t, ws, wx = state[i]
                # ws cols (h*8+d) = geh * wx  (fp16 2x; head 7 on gpsimd)
                for h in (7, 0, 1, 2, 3, 4, 5, 6):
                    eng = nc.vector if h < 7 else nc.gpsimd
                    eng.tensor_tensor(
                        out=_ap(ws[:], h * 8, [[72, R], [2, 4], [1, 2]]),
                        in0=_ap(geh_t[:], h * 8, [[64, R], [2, 4], [1, 2]]),
                        in1=_ap(wx[:], h * 2, [[16, R], [0, 4], [1, 2]]),
                        op=OP.mult,
                    )
                # PE: o1s[:, t*72 + c] = sum_gi ws[slot(t,gi)*72 + c]
                o1s = pp.tile([P, 72 * st], f32, tag="o1s")
                for t in range(st):
                    for gi in range(g):
                        s = t * g + gi
                        nc.tensor.matmul(
                            out=o1s[:, t * 72 : (t + 1) * 72],
                            lhsT=iden[:],
                            rhs=ws[:, s * 72 : (s + 1) * 72],
                            start=(gi == 0), stop=(gi == g - 1),
                        )
                state[i] = [o1s]

            def s2(i):
                """softmax normalize + ELU (z1 = elu(z)+1, b1 host-folded)."""
                (t0, st, g) = groups[i]
                (o1s,) = state[i]
                rec = wp.tile([P, 8 * st], f32, tag="rec")
                nc.vector.tensor_scalar_max(
                    out=_ap(rec[:], 0, [[8, st], [1, 8]]),
                    in0=_ap(o1s[:], 64, [[72, st], [1, 8]]),
                    scalar1=1e-30,
                )
                nc.vector.reciprocal(out=rec[:, 0 : 8 * st],
                                     in_=rec[:, 0 : 8 * st])
                z = wp.tile([P, 64 * st], f16, tag="z")
                nc.vector.tensor_tensor(
                    out=_ap(z[:], 0, [[64, st], [8, 8], [1, 8]]),
                    in0=_ap(o1s[:], 0, [[72, st], [8, 8], [1, 8]]),
                    in1=_ap(rec[:], 0, [[8, st], [1, 8], [0, 8]]),
                    op=OP.mult,
                )
                m = wp.tile([P, 64 * st], f16, tag="m")
                nc.vector.tensor_scalar_min(out=m[:, 0 : 64 * st],
                                            in0=z[:, 0 : 64 * st],
                                            scalar1=0.0)
                nc.scalar.activation(out=m[:, 0 : 64 * st],
                                     in_=m[:, 0 : 64 * st], func=AT.Exp)
                z1 = wp.tile([P, 64 * st], f16, tag="z1")
                nc.vector.scalar_tensor_tensor(
                    out=z1[:, 0 : 64 * st], in0=z[:, 0 : 64 * st], scalar=0.0,
                    in1=m[:, 0 : 64 * st], op0=OP.max, op1=OP.add,
                )
                state[i].append(z1)

            def s3(i):
                """h2 = z1 @ w2aug (transpose + matmul), convert, DMA out."""
                (t0, st, g) = groups[i]
                (_, z1) = state.pop(i)
                zTs = wp.tile([64, P * st], f16, tag="zTs")
                h2s = pp.tile([P, 66 * st], f32, tag="h2s")
                for t in range(st):
                    zT_ps = pt.tile([64, P], f16, tag="zT")
                    nc.tensor.transpose(out=zT_ps[:],
                                        in_=z1[:, t * 64 : (t + 1) * 64],
                                        identity=iden[:])
                    nc.scalar.copy(out=zTs[:, t * P : (t + 1) * P],
                                   in_=zT_ps[:])
                    nc.tensor.matmul(out=h2s[:, t * 66 : (t + 1) * 66],
                                     lhsT=zTs[:, t * P : (t + 1) * P],
                                     rhs=w2_t[:], start=True, stop=True)
                g2t = op_.tile([P, 66 * st], f16, tag="g2t")
                nc.vector.tensor_copy(g2t[:, 0 : st * 66],
                                      h2s[:, 0 : st * 66])
                nc.sync.dma_start(
                    out=bass.AP(tensor=out, offset=t0 * 66,
                                ap=[[NT * 66, P], [1, st * 66]]),
                    in_=g2t[:, 0 : st * 66],
                )

            n = len(groups)
            stages = [s0, s1, s2, s3]
            for k in range(n + len(stages) - 1):
                for j, fn in enumerate(stages):
                    i = k - j
                    if 0 <= i < n:
                        fn(i)
    nc.finalize()
    return nc


# ------------------------------------------------------------- launch C prog
def _build_C(groups):
    """Layer-2 edge pass + b2 + log_softmax -> [P, T*64] fp16."""
    nc = bacc.Bacc(None, target_bir_lowering=False)
    f16 = mybir.dt.float16
    f32 = mybir.dt.float32
    tot = int(sum(P * st * g for (_, st, g) in groups))
    geh = nc.dram_tensor("geh", [tot * 64], f16, kind="ExternalInput")
    gee = nc.dram_tensor("gee", [tot], f16, kind="ExternalInput")
    edst = nc.dram_tensor("edst", [P, NT * 2], f16, kind="ExternalInput")
    out = nc.dram_tensor("res", [P, NT * 64], f16, kind="ExternalOutput")

    AT = mybir.ActivationFunctionType
    OP = mybir.AluOpType
    with tile.TileContext(nc) as tc:
        with (
            tc.tile_pool(name="const", bufs=1) as cp,
            tc.tile_pool(name="gin", bufs=6) as gp,
            tc.tile_pool(name="work", bufs=4) as wp,
            tc.tile_pool(name="outp", bufs=1) as op_,
            tc.tile_pool(name="psum", bufs=3, space="PSUM") as pp,
        ):
            iden = cp.tile([P, P], f16, tag="iden")
            make_identity(nc, iden[:])
            edst_t = cp.tile([P, NT * 2], f16, tag="edst")
            nc.sync.dma_start(
                out=edst_t[:],
                in_=bass.AP(tensor=edst, offset=0,
                            ap=[[NT * 2, P], [1, NT * 2]]),
            )
            zball = op_.tile([P, NT * 64], f16, tag="zball")
            ssum = op_.tile([P, NT], f32, tag="ssum")

            offs = []
            oh = oe = 0
            for (t0, st, g) in groups:
                offs.append((oh, oe))
                oh += P * st * g * 64
                oe += P * st * g
            state = {}

            def s0(i):
                (t0, st, g) = groups[i]
                R = st * g
                off_h, off_e = offs[i]
                gee_t = gp.tile([P, STG_BUDGET], f16, tag="ge")
                nc.sync.dma_start(
                    out=gee_t[:, 0:R],
                    in_=bass.AP(tensor=gee, offset=off_e,
                                ap=[[R, P], [1, R]]),
                )
                geh_t = gp.tile([P, STG_BUDGET * 64], f16, tag="gh")
                nc.sync.dma_start(
                    out=geh_t[:, 0 : R * 64],
                    in_=bass.AP(tensor=geh, offset=off_h,
                                ap=[[R * 64, P], [1, R * 64]]),
                )
                es = wp.tile([P, STG_BUDGET], f16, tag="es")
                nc.gpsimd.tensor_tensor(
                    out=_ap(es[:], 0, [[g, st], [2, g // 2], [1, 2]]),
                    in0=_ap(gee_t[:], 0, [[g, st], [2, g // 2], [1, 2]]),
                    in1=_ap(edst_t[:], t0 * 2, [[2, st], [0, g // 2], [1, 2]]),
                    op=OP.add,
                )
                lr = wp.tile([P, STG_BUDGET], f16, tag="lr")
                nc.scalar.activation(out=lr[:, 0:R], in_=es[:, 0:R],
                                     func=AT.Prelu, alpha=0.2)
                ws = wp.tile([P, STG_BUDGET * 65], f16, tag="ws")
                nc.scalar.activation(
                    out=_ap(ws[:], 64, [[65, R]]),
                    in_=_ap(lr[:], 0, [[1, R]]),
                    func=AT.Exp,
                )
                wx = wp.tile([P, STG_BUDGET * 2], f16, tag="wx")
                nc.scalar.activation(
                    out=_ap(wx[:], 0, [[2, R], [1, 2]]),
                    in_=_ap(lr[:], 0, [[1, R], [0, 2]]),
                    func=AT.Exp,
                )
                state[i] = [geh_t, ws, wx]

            def s1(i):
                (t0, st, g) = groups[i]
                R = st * g
                geh_t, ws, wx = state[i]
                # ws cols 0..63 = geh * wx; split slots 7:1 DVE:Pool
                Rd = (R * 7 // 8) & ~1
                nc.gpsimd.tensor_tensor(
                    out=_ap(ws[:], Rd * 65, [[65, R - Rd], [2, 32], [1, 2]]),
                    in0=_ap(geh_t[:], Rd * 64, [[64, R - Rd], [2, 32], [1, 2]]),
                    in1=_ap(wx[:], Rd * 2, [[2, R - Rd], [0, 32], [1, 2]]),
                    op=OP.mult,
                )
                nc.vector.tensor_tensor(
                    out=_ap(ws[:], 0, [[65, Rd], [2, 32], [1, 2]]),
                    in0=_ap(geh_t[:], 0, [[64, Rd], [2, 32], [1, 2]]),
                    in1=_ap(wx[:], 0, [[2, Rd], [0, 32], [1, 2]]),
                    op=OP.mult,
                )
                o1s = pp.tile([P, 65 * st], f32, tag="o1s")
                for t in range(st):
                    for gi in range(g):
                        s = t * g + gi
                        nc.tensor.matmul(
                            out=o1s[:, t * 65 : (t + 1) * 65],
                            lhsT=iden[:],
                            rhs=ws[:, s * 65 : (s + 1) * 65],
                            start=(gi == 0), stop=(gi == g - 1),
                        )
                state[i] = [o1s]

            def s2(i):
                (t0, st, g) = groups[i]
                (o1s,) = state.pop(i)
                rec = wp.tile([P, st], f32, tag="rec")
                nc.vector.tensor_scalar_max(
                    out=rec[:, 0:st],
                    in0=_ap(o1s[:], 64, [[65, st]]),
                    scalar1=1e-30,
                )
                nc.vector.reciprocal(out=rec[:, 0:st], in_=rec[:, 0:st])
                # z (b2 folded into the gather table on host) -> zball
                nc.vector.tensor_tensor(
                    out=_ap(zball[:], t0 * 64, [[64, st], [1, 64]]),
                    in0=_ap(o1s[:], 0, [[65, st], [1, 64]]),
                    in1=_ap(rec[:], 0, [[1, st], [0, 64]]),
                    op=OP.mult,
                )
                ex = wp.tile([P, 64 * st], f16, tag="ex")
                nc.scalar.activation(
                    out=ex[:, 0 : 64 * st],
                    in_=_ap(zball[:], t0 * 64, [[64, st], [1, 64]]),
                    func=AT.Exp,
                )
                nc.vector.reduce_sum(
                    out=_ap(ssum[:], t0, [[1, st]]),
                    in_=_ap(ex[:], 0, [[64, st], [1, 64]]),
                    axis=mybir.AxisListType.X,
                )

            n = len(groups)
            stages = [s0, s1, s2]
            for k in range(n + len(stages) - 1):
                for j, fn in enumerate(stages):
                    i = k - j
                    if 0 <= i < n:
                        fn(i)
            # log_softmax tail: lse = ln(ssum) once, out = zb - lse
            lse = cp.tile([P, NT], f32, tag="lse")
            nc.scalar.activation(out=lse[:], in_=ssum[:], func=AT.Ln)
            ot = op_.tile([P, NT * 64], f16, tag="ot")
            nch = NT // 4
            for i in range(4):
                lo = i * nch
                hi = NT if i == 3 else (i + 1) * nch
                eng = nc.vector if i % 2 == 0 else nc.gpsimd
                eng.tensor_tensor(
                    out=_ap(ot[:], lo * 64, [[64, hi - lo], [1, 64]]),
                    in0=_ap(zball[:], lo * 64, [[64, hi - lo], [1, 64]]),
                    in1=_ap(lse[:], lo, [[1, hi - lo], [0, 64]]),
                    op=OP.subtract,
                )
                nc.sync.dma_start(
                    out=bass.AP(tensor=out, offset=lo * 64,
                                ap=[[NT * 64, P], [1, (hi - lo) * 64]]),
                    in_=ot[:, lo * 64 : hi * 64],
                )
    nc.finalize()
    return nc


# ------------------------------------------------------------------- driver
def _get_programs(groups):
    key = tuple(groups)
    if key not in _cache:
        _cache[key] = (_build_A(), _build_B(groups), _build_C(groups))
    return _cache[key]


def kernel(x, edge_index, W1, att_src1, att_dst1, b1, W2, att_src2, att_dst2, b2,
           _timings=None):
    import time as _time

    x = np.asarray(x, dtype=np.float32)
    W1 = np.asarray(W1, dtype=np.float32)
    order_all, pos, groups, A, tot = _host_prep(np.asarray(edge_index))
    ncA, ncB, ncC = _get_programs(groups)

    # ---- launch A inputs: W1aug = [W1 | W1@Asrc | W1@Adst]
    W1r = W1.reshape(F_IN, 8, 8)
    w1s = np.einsum("khd,hd->kh", W1r, np.asarray(att_src1, np.float32))
    w1d = np.einsum("khd,hd->kh", W1r, np.asarray(att_dst1, np.float32))
    w1aug = np.concatenate([W1, w1s, w1d], axis=1).astype(np.float16)
    xpad = np.vstack([x, np.zeros((1, F_IN), np.float32)])
    in_A = []
    for c in range(NCORES):
        xa = xpad[np.where(order_all[c] >= 0, order_all[c], N)]  # [NPAD, 300]
        in_A.append({"xT": np.ascontiguousarray(xa.T, dtype=np.float16),
                     "w1": w1aug})

    t0 = _time.perf_counter()
    resA = run_bass_kernel_spmd(ncA, in_A, core_ids=list(range(NCORES)))
    tA = _time.perf_counter() - t0

    # h1x per core: [P, T*80] fp16 -> tables in pi order.
    # b1 is folded into the h table: sum(alpha)=1 so the post-aggregation
    # bias is equivalent to biasing every gathered row.
    b1f = np.asarray(b1, np.float32)
    h1x3 = [r["h1x"].reshape(P, NT, 80) for r in resA.results]
    tab1h = np.vstack(
        [(h[:, :, :64].transpose(1, 0, 2).reshape(NPAD, 64) + b1f)
         .astype(np.float16) for h in h1x3]
        + [np.zeros((1, 64), np.float16)])
    tab1e = np.vstack(
        [h[:, :, 64:72].transpose(1, 0, 2).reshape(NPAD, 8) for h in h1x3]
        + [np.full((1, 8), SENT_BIG, np.float16)])

    # ---- launch B inputs
    W2 = np.asarray(W2, np.float32)
    w2aug = np.concatenate(
        [W2, (W2 @ np.asarray(att_src2, np.float32).ravel())[:, None],
         (W2 @ np.asarray(att_dst2, np.float32).ravel())[:, None]], axis=1)
    badj = -w2aug.sum(axis=0).astype(np.float32)
    b1 = np.asarray(b1, np.float16)
    in_B = []
    for c in range(NCORES):
        in_B.append({"geh": tab1h[A[c]].ravel(),
                     "gee": tab1e[A[c]].ravel(),
                     "edst": np.ascontiguousarray(
                         h1x3[c][:, :, 72:80].reshape(P, NT * 8)),
                     "w2aug": w2aug.astype(np.float16)})

    t0 = _time.perf_counter()
    resB = run_bass_kernel_spmd(ncB, in_B, core_ids=list(range(NCORES)))
    tB = _time.perf_counter() - t0

    # g2 arrives without badj (host-folded); b2 also folds into the h table
    badj16 = badj.astype(np.float32)
    b2f = np.asarray(b2, np.float32)
    g23 = [r["g2"].reshape(P, NT, 66) for r in resB.results]
    tab2h = np.vstack(
        [(g[:, :, :64].transpose(1, 0, 2).reshape(NPAD, 64).astype(np.float32)
          + badj16[:64] + b2f).astype(np.float16) for g in g23]
        + [np.zeros((1, 64), np.float16)])
    tab2e = np.vstack(
        [(g[:, :, 64:65].transpose(1, 0, 2).reshape(NPAD, 1).astype(np.float32)
          + badj16[64]).astype(np.float16) for g in g23]
        + [np.full((1, 1), SENT_BIG, np.float16)])

    # ---- launch C inputs
    in_C = []
    for c in range(NCORES):
        ed = g23[c][:, :, 65:66].astype(np.float32) + badj16[65]  # [P, T, 1]
        ed = np.repeat(ed, 2, axis=2).astype(np.float16)          # [P, T, 2]
        in_C.append({"geh": tab2h[A[c]].ravel(),
                     "gee": tab2e[A[c]].ravel(),
                     "edst": np.ascontiguousarray(ed.reshape(P, NT * 2))})

    t0 = _time.perf_counter()
    resC = run_bass_kernel_spmd(ncC, in_C, core_ids=list(range(NCORES)))
    tC = _time.perf_counter() - t0

    out = np.empty((N, 64), np.float32)
    for c in range(NCORES):
        res = resC.results[c]["res"].reshape(P, NT, 64)
        res = res.transpose(1, 0, 2).reshape(NPAD, 64).astype(np.float32)
        valid = order_all[c] >= 0
        out[order_all[c][valid]] = res[valid]
    if _timings is not None:
        _timings.update({"A": tA, "B": tB, "C": tC})
    return out
